# revision 35
# baseline (speedup 1.0000x reference)
"""AnyStory Flux attention processor on 8 TRN2 NeuronCores.

Sharding: tensor-parallel over heads (24 heads -> 3 per core), no
collectives; the host gathers along the head axis and performs the final
softmax normalization (divide by the ones-column sums) while unsharding.

Device algorithm per head (S=3168 = 512 txt + 64 redux + 2048 img +
512 ref + 32 router; D=128), in ST orientation (k on partitions, q free):

  seg1 (q 0:2624 x k 0:3136+pad): QK and PV in bf16 (fp8 variants were
    measured on host: one-sided e4m3 QK noise gives rel-err 2.9e-2 > 2e-2,
    and fully compensated hi+lo fp8 costs parity with bf16 - no win).
    The softmax exp is split across the two PSUM-capable elementwise
    engines at single k-tile granularity (25 ops per q-block) so the
    PSUM st tiles are one bank each and the QK->exp->PV software
    pipeline can run 6 deep (kills PE stalls on st-buffer recycling):
      ACT tiles (13): true exp -> bf16 p (bias aligns the scale to the
        DVE tiles' Schraudolph factor).
      DVE tiles (12, including every masked tile): Schraudolph exp in
        ONE fused tensor op: i16 = round(184.665*x + 16250.43), bitcast
        = bf16 ~ e^x (+-3%); the additive attention masks ride the same
        instruction as the int16 tensor operand.
  seg2: per-cond ref self-attention, plain bf16 (peaked softmax).
  seg3: router q x [img ; router] keys, bf16, exp on ACT.

  (numerator | denominator) leave PSUM as f16 via ACT/DVE copies
  (alternating) and are DMAed; the host divides while unsharding.
  No max-subtraction anywhere (|logit| <~ 6.6, masks <= 1.5).

DMA orchestration (the cost model serializes HWDGE at 625ns/DMA and
holds the issuing SEQ during a DMA's waits):
  - head-0 inputs are split across two independent DMA paths in strict
    JIT consumption order: kt/qt/am on the SP queue (HWDGE), v1/v2/v3 on
    the gpsimd queue (SWDGE, bypasses HWDGE);
  - output DMAs go on the gpsimd queue (program-order after all gpsimd
    input DMAs) so they never head-of-line-block input dispatch, except
    the final tail blocks which use the then-idle SP queue;
  - head 2's seg2/seg3 run at the very end so the kernel tail is the
    tiny router block;
  - a longer PE warmup (56 matmuls) covers the first-chunk DMA latency
    and finishes the p-state ramp right as real work begins.
"""

import math
import numpy as np
import ml_dtypes
from contextlib import ExitStack

import concourse.bass as bass
import concourse.tile as tile
from concourse import mybir, bacc
from concourse.bass_utils import run_bass_kernel_spmd

# ---- problem constants (hardcoded; kernel.py must be self-contained)
B, H, D = 1, 24, 128
TXT, REDUX, IMG, REF, ROUTER, NCOND = 512, 64, 2048, 512, 32, 2
S = TXT + REDUX + IMG + REF + ROUTER          # 3168
TE = TXT                                       # 512
TRE = TE + REDUX                               # 576
TRI = TRE + IMG                                # 2624
TRIR = TRI + REF                               # 3136
REF_SHIFT = 1.5
SP = 3200                                      # padded key length (25*128)
NKT = SP // 128                                # 25 seg1 k-tiles
HPC = H // 8                                   # heads per core = 3

SQ = 1.0 / math.sqrt(128.0)
A16 = 128.0 / math.log(2.0)                    # 184.664965
B16 = 16250.43                                 # 127*128 - 5.57 (Schraudolph)
# ACT tiles must carry the same scale factor as the Schraudolph tiles:
# bitcast(round(A16*x + B16)) ~ e^x * 2^((B16-16256)/128) * g, E[g]=1.0298
ACT_BIAS = (B16 - 16256.0) / 128.0 * math.log(2.0) + math.log(1.0298)
MASK_CLAMP = -20.0

F32 = mybir.dt.float32
F16 = mybir.dt.float16
BF16 = mybir.dt.bfloat16
I16 = mybir.dt.int16
EXP = mybir.ActivationFunctionType.Exp
COPY = mybir.ActivationFunctionType.Copy
MULT = mybir.AluOpType.mult
ADD = mybir.AluOpType.add

QBLOCKS = [(0, 384), (384, 384), (768, 384), (1152, 384),
           (1536, 384), (1920, 384), (2304, 320)]
# single k-tile work units: (tile, engine, mask_slot); ACT owns tiles
# 0-13 minus 4, DVE owns 14-24 plus 4 (all masked tiles are DVE - the
# fused mask op is a DVE op). Interleave keeps both engines fed; masked
# tiles sit late so their am16 chunks have streamed in.
TILES = [(0, "A", None), (1, "A", None), (2, "A", None), (3, "A", None),
         (14, "D", None), (15, "D", None), (16, "D", None), (17, "D", None),
         (5, "A", None), (6, "A", None), (18, "D", None), (7, "A", None),
         (19, "D", None), (8, "A", None), (4, "D", 0), (9, "A", None),
         (20, "D", 1), (10, "A", None), (21, "D", 2), (11, "A", None),
         (22, "D", 3), (12, "A", None), (23, "D", 4), (13, "A", None),
         (24, "D", 5)]

WARMN = 47          # PE warmup matmuls (64 cols each)
PIPE = 7            # PV lags QK by this many work items
FIN_DEFER = 5       # finalize copies deferred this many items

# head-0 SP-queue (HWDGE) chunk schedule, strict JIT consumption order
H0_SP = [
    ("kt", 0, 512), ("kt", 1792, 2304), ("kt", 512, 1152),
    ("kt", 2304, 2688), ("kt", 1152, 1792), ("kt", 2688, 3200),
    ("am", 0, 2, 0, 384),
    ("am", 2, 4, 0, 384), ("am", 4, 6, 0, 384), ("qt", 384, 768),
    ("am", 0, 2, 384, 1152), ("am", 2, 4, 384, 1152), ("am", 4, 6, 384, 1152),
    ("qt", 768, 1152), ("qt2",), ("kt2",), ("qt", 1152, 1536),
    ("am", 0, 2, 1152, 1920), ("am", 2, 4, 1152, 1920), ("am", 4, 6, 1152, 1920),
    ("qt", 1536, 1920), ("qt", 1920, 2304),
    ("am", 0, 2, 1920, 2624), ("am", 2, 4, 1920, 2624), ("am", 4, 6, 1920, 2624),
    ("qt", 2304, 2624), ("qt", 3136, 3168),
]
# head-0 gpsimd-queue (SWDGE) chunk schedule (v tiles in consumption order;
# qt block 0 rides here so it lands in parallel with SP's first kt chunk)
H0_POOL = [("qt", 0, 384), ("v1", 0, 4), ("v1", 14, 18), ("v1", 4, 9),
           ("v1", 18, 22), ("v1", 9, 14), ("v1", 22, 25), ("v2",), ("v3",),
           ("eye",)]


def _subs(qw):
    out, o = [], 0
    while o < qw:
        w = min(128, qw - o)
        out.append((o, w))
        o += w
    return out


def build_nc():
    nc = bacc.Bacc()
    kt_d = nc.declare_dram_parameter("kt", [HPC, 128, SP], BF16, isOutput=False)
    qt_d = nc.declare_dram_parameter("qt", [HPC, 128, S], BF16, isOutput=False)
    # V pre-tiled partition-major [128, T, 129] (value | ones)
    v1_d = nc.declare_dram_parameter("v1", [HPC, 128, NKT, 129], BF16, isOutput=False)
    v2_d = nc.declare_dram_parameter("v2", [HPC, 128, 4, 129], BF16, isOutput=False)
    v3_d = nc.declare_dram_parameter("v3", [HPC, 128, 17, 129], BF16, isOutput=False)
    qt2_d = nc.declare_dram_parameter("qt2", [HPC, 128, 512], BF16, isOutput=False)
    kt2_d = nc.declare_dram_parameter("kt2", [HPC, 128, 512], BF16, isOutput=False)
    # fused-mask payloads: slots for tiles (4, 20, 21, 22, 23, 24)
    am_d = nc.declare_dram_parameter("am16", [128, 6, TRI], I16, isOutput=False)
    eye_d = nc.declare_dram_parameter("eye", [128, 128], F16, isOutput=False)
    out_d = nc.declare_dram_parameter("out", [HPC, S, 129], F16, isOutput=True)
    out_f = out_d.rearrange("h s d -> (h s) d")

    with ExitStack() as ctx:
        tc = ctx.enter_context(tile.TileContext(nc))
        const = ctx.enter_context(tc.tile_pool(name="const", bufs=1))
        stp = ctx.enter_context(tc.tile_pool(name="st", bufs=5, space="PSUM"))
        accp = ctx.enter_context(tc.tile_pool(name="acc", bufs=3, space="PSUM"))
        ptp = ctx.enter_context(tc.tile_pool(name="pt", bufs=10))
        stgp = ctx.enter_context(tc.tile_pool(name="stg", bufs=10))
        smallp = ctx.enter_context(tc.tile_pool(name="small", bufs=4))

        # warmup weights memset FIRST (on DVE - gpsimd would delay the Pool
        # queue's first input DMA) so PE can start spinning immediately
        warm = const.tile([128, 64], BF16, tag="warm")
        nc.vector.memset(warm[:, :], 0.0)

        am_sb = const.tile([128, 6, TRI], I16, tag="am16")
        bias_sb = const.tile([128, 1], F32, tag="biasA")
        zbias_sb = const.tile([128, 1], F32, tag="biasZ")
        eye_sb = const.tile([128, 128], F16, tag="eye")
        ones_sb = const.tile([128, 1], BF16, tag="ones")
        nc.vector.memset(bias_sb[:, :], ACT_BIAS)
        nc.vector.memset(zbias_sb[:, :], 0.0)
        nc.vector.memset(ones_sb[:, :], 1.0)

        kt_sb, qt_sb, v1_sb = [], [], []
        qt2_sb, kt2_sb, v2_sb, v3_sb = [], [], [], []
        load_fns = []
        for h in range(HPC):
            kt = const.tile([128, SP], BF16, tag=f"kt{h}")
            qt = const.tile([128, S], BF16, tag=f"qt{h}")
            v1 = const.tile([128, NKT, 129], BF16, tag=f"v1{h}")
            qt2 = const.tile([128, 512], BF16, tag=f"qt2{h}")
            kt2 = const.tile([128, 512], BF16, tag=f"kt2{h}")
            v2 = const.tile([128, 4, 129], BF16, tag=f"v2{h}")
            v3 = const.tile([128, 17, 129], BF16, tag=f"v3{h}")
            kt_sb.append(kt); qt_sb.append(qt); v1_sb.append(v1)
            qt2_sb.append(qt2); kt2_sb.append(kt2)
            v2_sb.append(v2); v3_sb.append(v3)

            def load_head(h=h, kt=kt, qt=qt, v1=v1, qt2=qt2, kt2=kt2,
                          v2=v2, v3=v3):
                def chunk(eng, c):
                    kind = c[0]
                    if kind == "kt":
                        eng.dma_start(kt[:, c[1]:c[2]], kt_d[h, :, c[1]:c[2]])
                    elif kind == "qt":
                        eng.dma_start(qt[:, c[1]:c[2]], qt_d[h, :, c[1]:c[2]])
                    elif kind == "am":
                        eng.dma_start(am_sb[:, c[1]:c[2], c[3]:c[4]],
                                      am_d[:, c[1]:c[2], c[3]:c[4]])
                    elif kind == "v1":
                        eng.dma_start(v1[:, c[1]:c[2], :], v1_d[h, :, c[1]:c[2], :])
                    elif kind == "qt2":
                        eng.dma_start(qt2[:, :], qt2_d[h])
                    elif kind == "kt2":
                        eng.dma_start(kt2[:, :], kt2_d[h])
                    elif kind == "v2":
                        eng.dma_start(v2[:, :, :], v2_d[h])
                    elif kind == "v3":
                        eng.dma_start(v3[:, :, :], v3_d[h])
                    elif kind == "eye":
                        eng.dma_start(eye_sb[:, :], eye_d[:, :])

                if h == 0:
                    for c in H0_SP:
                        chunk(nc.sync, c)
                    for c in H0_POOL:
                        chunk(nc.gpsimd, c)
                else:
                    for c in range(4):
                        nc.sync.dma_start(kt[:, c * 800:(c + 1) * 800],
                                          kt_d[h, :, c * 800:(c + 1) * 800])
                    for c in [(0, 792), (792, 1584), (1584, 2376),
                              (2376, 2624), (3136, 3168)]:
                        nc.sync.dma_start(qt[:, c[0]:c[1]], qt_d[h, :, c[0]:c[1]])
                    nc.sync.dma_start(qt2[:, :], qt2_d[h])
                    nc.sync.dma_start(kt2[:, :], kt2_d[h])
                    nc.gpsimd.dma_start(v1[:, :, :], v1_d[h])
                    nc.gpsimd.dma_start(v2[:, :, :], v2_d[h])
                    nc.gpsimd.dma_start(v3[:, :, :], v3_d[h])
            load_fns.append(load_head)

        # ---- PE warmup: spin the Tensor engine to full p-state during the
        # initial DMA wait (the cost model ramps PE clock 0.65->2.4 GHz over
        # ~3us of continuous execution)
        wst = stp.tile([128, 512], F32, tag="st", name="st")
        for wi in range(WARMN):
            nc.tensor.matmul(wst[0:64, 0:64], lhsT=warm[:, :],
                             rhs=warm[:, :], start=True, stop=True)

        # ---- work items (global PIPE-deep software pipeline)
        items = []
        copy_flip = [0]

        def finalize(acc, h, q0, qw, eng=None):
            """Copy acc PSUM -> f16 stage (ACT/DVE alternating), DMA out."""
            if eng is None:
                eng = nc.gpsimd
            subs = _subs(qw)
            stg = stgp.tile([128, 3, 129], F16, tag="stg", name="stg")
            on_act = (copy_flip[0] % 2 == 0)
            copy_flip[0] += 1
            nsub = len(subs)
            if all(wsub == 128 for _, wsub in subs):
                src = acc[:, 0:nsub * 129].rearrange("p (a b) -> p a b", a=nsub)
                if on_act:
                    nc.scalar.activation(stg[:, 0:nsub, :], src, COPY)
                else:
                    nc.vector.tensor_scalar_add(stg[:, 0:nsub, :], src, 0.0)
            else:
                # ragged tail: avoid reading never-written PSUM
                w_full = (nsub - 1) * 129
                lastw = subs[-1][1]
                if nsub > 1:
                    src = acc[:, 0:w_full].rearrange("p (a b) -> p a b", a=nsub - 1)
                    if on_act:
                        nc.scalar.activation(stg[:, 0:nsub - 1, :], src, COPY)
                    else:
                        nc.vector.tensor_scalar_add(stg[:, 0:nsub - 1, :], src, 0.0)
                if on_act:
                    nc.scalar.activation(stg[0:lastw, nsub - 1, :],
                                         acc[0:lastw, w_full:w_full + 129], COPY)
                else:
                    nc.vector.tensor_scalar_add(stg[0:lastw, nsub - 1, :],
                                                acc[0:lastw, w_full:w_full + 129], 0.0)
            r0 = h * S + q0
            nfull = qw // 128
            if nfull:
                dst = out_f[r0:r0 + nfull * 128, :].rearrange(
                    "(si p) d -> p si d", si=nfull)
                eng.dma_start(dst, stg[:, 0:nfull, :])
            if qw % 128:
                lw = qw % 128
                eng.dma_start(out_f[r0 + nfull * 128:r0 + qw, :],
                              stg[0:lw, nfull, :])

        for h in range(HPC):
            load_fns[h]()
            kt, qt, v1 = kt_sb[h], qt_sb[h], v1_sb[h]
            head_items = []

            for qbi, (q0, qw) in enumerate(QBLOCKS):
                subs = _subs(qw)
                blk = {}

                def qk1(st, t, kt=kt, qt=qt, q0=q0, qw=qw):
                    nc.tensor.matmul(
                        st[:, 0:qw],
                        lhsT=kt[:, t * 128:(t + 1) * 128],
                        rhs=qt[:, q0:q0 + qw],
                        start=True, stop=True)

                def ex1(st, t, eng, mslot, q0=q0, qw=qw):
                    pt = ptp.tile([128, 512], BF16, tag="pt", name="pt")
                    if eng == "A":
                        nc.scalar.activation(pt[:, 0:qw], st[:, 0:qw],
                                             EXP, bias=bias_sb[:, :], scale=1.0)
                    elif mslot is None:
                        nc.vector.tensor_scalar(
                            pt[:, 0:qw].bitcast(I16),
                            st[:, 0:qw], A16, B16, MULT, ADD)
                    else:
                        nc.vector.scalar_tensor_tensor(
                            pt[:, 0:qw].bitcast(I16),
                            st[:, 0:qw], A16,
                            am_sb[:, mslot, q0:q0 + qw], MULT, ADD)
                    return pt

                def pv1(pt, t, last, h=h, v1=v1, q0=q0, qw=qw, subs=subs, blk=blk):
                    if "acc" not in blk:
                        blk["acc"] = accp.tile([128, 512], F32, tag="acc", name="acc")
                        blk["n"] = 0
                    acc = blk["acc"]
                    for si, (qs0, qsw) in enumerate(subs):
                        nc.tensor.matmul(
                            acc[0:qsw, si * 129:si * 129 + 129],
                            lhsT=pt[:, qs0:qs0 + qsw],
                            rhs=v1[:, t, :],
                            start=(blk["n"] == 0),
                            stop=(last and si == len(subs) - 1))
                        blk["n"] += 1
                    if last:
                        # last block of the last head drains via the idle SP
                        # queue so the kernel tail isn't swdge-latency-bound
                        eng = nc.sync if (h == HPC - 1 and q0 + qw == TRI) else None
                        return lambda: finalize(acc, h, q0, qw, eng)

                for ti, (t, eng, mslot) in enumerate(TILES):
                    last = (ti == len(TILES) - 1)
                    head_items.append((
                        (lambda st, t=t, f=qk1: f(st, t)),
                        (lambda st, t=t, e=eng, m=mslot, f=ex1: f(st, t, e, m)),
                        (lambda pt, t=t, l=last, f=pv1: f(pt, t, l)),
                    ))

            # ===== seg2: per-cond ref self-attention (bf16) =====
            seg23_items = []
            last_head = (h == HPC - 1)
            for c in range(NCOND):
                b0 = 256 * c

                def qk2(st, h=h, b0=b0):
                    for j in range(2):
                        nc.tensor.matmul(
                            st[:, j * 256:(j + 1) * 256],
                            lhsT=kt2_sb[h][:, b0 + j * 128:b0 + (j + 1) * 128],
                            rhs=qt2_sb[h][:, b0:b0 + 256],
                            start=True, stop=True)

                def ex2(st, h=h):
                    pt = smallp.tile([128, 512], BF16, tag="pt2", name="pt2")
                    nc.scalar.activation(pt[:, 0:512], st[:, 0:512],
                                         EXP, bias=zbias_sb[:, :], scale=1.0)
                    return pt

                def pv2(pt, h=h, b0=b0, c=c, lh=last_head):
                    acc = accp.tile([128, 512], F32, tag="acc", name="acc")
                    for j in range(2):
                        for si in range(2):
                            nc.tensor.matmul(
                                acc[0:128, si * 129:si * 129 + 129],
                                lhsT=pt[:, j * 256 + si * 128:j * 256 + (si + 1) * 128],
                                rhs=v2_sb[h][:, 2 * c + j, :],
                                start=(j == 0 and si == 0),
                                stop=(j == 1 and si == 1))
                    return lambda: finalize(acc, h, TRI + b0, 256)

                seg23_items.append((qk2, ex2, pv2))

            # ===== seg3: router queries =====
            def qk3(st, h=h, kt=kt, qt=qt):
                # st is a pair (img-part tile, router-self tile)
                st_a, st_b = st
                for i in range(16):
                    nc.tensor.matmul(
                        st_a[:, i * 32:(i + 1) * 32],
                        lhsT=kt[:, TRE + i * 128:TRE + (i + 1) * 128],
                        rhs=qt[:, TRIR:TRIR + 32],
                        start=(i == 0), stop=(i == 15))
                nc.tensor.matmul(
                    st_b[0:32, 0:32],
                    lhsT=kt[:, TRIR:TRIR + 32],
                    rhs=qt[:, TRIR:TRIR + 32],
                    start=True, stop=True)

            def ex3(st, h=h):
                st_a, st_b = st
                pt = smallp.tile([128, 512], BF16, tag="pt3", name="pt3")
                ptb = smallp.tile([128, 32], BF16, tag="pt3b", name="pt3b")
                nc.scalar.activation(pt[:, 0:512], st_a[:, 0:512],
                                     EXP, bias=zbias_sb[:, :], scale=1.0)
                nc.scalar.activation(ptb[0:32, 0:32], st_b[0:32, 0:32],
                                     EXP, bias=zbias_sb[0:32, :], scale=1.0)
                return (pt, ptb)

            def pv3(pts, h=h, lh=last_head):
                # transposed PV: v stationary, p moving -> numerator lands as
                # [128 d, 32 q] (544 streamed cols instead of 17*129); the
                # denominator comes from 1-col ones-contractions; a PE
                # transpose via the identity then restores [32 q, 128 d].
                # num (f32 cols 0:32), den (col 40) and the f16 transpose
                # scratch (f32 cols 128:192) share one PSUM bank: the bank is
                # zeroed by the first matmul's start, and later start=False
                # writes to untouched regions land on hardware-zeroed bytes.
                pt, ptb = pts
                acc = accp.tile([128, 512], F32, tag="acc", name="acc")
                for i in range(16):
                    nc.tensor.matmul(
                        acc[0:128, 0:32],
                        lhsT=v3_sb[h][:, i, 0:128],
                        rhs=pt[:, i * 32:(i + 1) * 32],
                        start=(i == 0), stop=False)
                nc.tensor.matmul(
                    acc[0:128, 0:32],
                    lhsT=v3_sb[h][0:32, 16, 0:128],
                    rhs=ptb[0:32, 0:32],
                    start=False, stop=False, skip_group_check=True)
                for i in range(16):
                    nc.tensor.matmul(
                        acc[0:32, 40:41],
                        lhsT=pt[:, i * 32:(i + 1) * 32],
                        rhs=ones_sb[:, 0:1],
                        start=False, stop=False, skip_group_check=True)
                nc.tensor.matmul(
                    acc[0:32, 40:41],
                    lhsT=ptb[0:32, 0:32],
                    rhs=ones_sb[0:32, 0:1],
                    start=False, stop=True, skip_group_check=True)

                def fin(acc=acc, h=h, lh=lh):
                    tb = smallp.tile([128, 32], F16, tag="tb", name="tb")
                    nc.scalar.activation(tb[:, 0:32], acc[0:128, 0:32], COPY)
                    ot = acc[:, 128:192].bitcast(F16)      # [128, 128] f16
                    nc.tensor.matmul(ot[0:32, 0:128], lhsT=tb[:, 0:32],
                                     rhs=eye_sb[:, 0:128], is_transpose=True,
                                     start=False, stop=True,
                                     skip_group_check=True)
                    stg = stgp.tile([128, 3, 129], F16, tag="stg", name="stg")
                    nc.vector.tensor_scalar_add(stg[0:32, 0, 0:128],
                                                ot[0:32, 0:128], 0.0)
                    nc.scalar.activation(stg[0:32, 0, 128:129],
                                         acc[0:32, 40:41], COPY)
                    r0 = h * S + TRIR
                    eng = nc.sync if lh else nc.gpsimd
                    eng.dma_start(out_f[r0:r0 + 32, :], stg[0:32, 0, :])
                return fin

            if last_head:
                # tail: weave seg2 into the last q-block so its exps overlap
                # seg1 PE work; seg3 runs dead last so the kernel ends on the
                # tiny router block's DMA. seg3's QK+exp issue ~8 items early
                # so its PV never waits on the ACT queue at the stream end.
                pt3_cell = []

                def qk_ex3(st, f1=qk3, f2=ex3, cell=pt3_cell):
                    f1(st)
                    cell.append(f2(st))

                def pv3_late(_pt, f=pv3, cell=pt3_cell):
                    return f(cell[0])

                head_items.insert(162, seg23_items[0])
                head_items.insert(169, (qk_ex3, lambda st: None,
                                        lambda pt: None, "seg3"))
                head_items.insert(174, seg23_items[1])
                head_items.append((lambda st: None, lambda st: None,
                                   pv3_late))
            else:
                # splice into the middle of the head's stream so their small
                # bursty windows don't cluster at head boundaries
                seg23_items.append((qk3, ex3, pv3, "seg3"))
                for i, it in enumerate(seg23_items):
                    head_items.insert(58 + i * 25, it)
            items.extend(head_items)

        # ---- run the global pipeline; finalize copies are deferred a few
        # items so they never sit in an exp engine's queue ahead of work the
        # PE pipeline depends on
        pending, fins = [], []
        idx = 0
        nitems = len(items)
        for it in items:
            fqk, fex, fpv = it[0], it[1], it[2]
            is_seg3 = len(it) > 3
            while fins and fins[0][1] <= idx:
                fins.pop(0)[0]()
            if is_seg3:
                st = (stp.tile([128, 512], F32, tag="st", name="st"),
                      stp.tile([128, 512], F32, tag="st", name="st"))
            else:
                st = stp.tile([128, 512], F32, tag="st", name="st")
            fqk(st)
            while len(pending) >= PIPE:
                fin = pending.pop(0)()
                if fin is not None:
                    fins.append((fin, idx + FIN_DEFER))
            pt = fex(st)
            pending.append(lambda f=fpv, p=pt: f(p))
            idx += 1
        while pending:
            while fins and fins[0][1] <= idx:
                fins.pop(0)[0]()
            fin = pending.pop(0)()
            if fin is not None:
                fins.append((fin, idx))
            idx += 1
        while fins:
            fins.pop(0)[0]()

    nc.compile()
    return nc


_NC_CACHE = None


def _get_nc():
    global _NC_CACHE
    if _NC_CACHE is None:
        _NC_CACHE = build_nc()
    return _NC_CACHE


def make_in_maps(query, key, value, ref_mask, routing_map):
    q = np.asarray(query, np.float32)[0]                  # [24, S, 128]
    k = np.asarray(key, np.float32)[0]
    v = np.asarray(value, np.float32)[0]
    rm = np.asarray(ref_mask, np.float32)[0]              # [512, 2624]
    rt = np.asarray(routing_map, np.float32)[0]           # [2, 2048]

    qt = np.ascontiguousarray(
        (q * SQ).transpose(0, 2, 1)).astype(ml_dtypes.bfloat16)   # [24,128,S]
    ktf = np.zeros((H, 128, SP), np.float32)
    ktf[:, :, :S] = k.transpose(0, 2, 1)
    kt = ktf.astype(ml_dtypes.bfloat16)

    # V (+ones) pre-tiled partition-major [128, T, 129]
    vv = np.zeros((H, SP, 129), np.float32)
    vv[:, :S, :128] = v
    vv[:, :TRIR, 128] = 1.0                               # ones: seg1 keys only
    vv[:, 24 * 128 + 64:] = 0.0                           # router+pad rows
    v1 = np.ascontiguousarray(
        vv.reshape(H, NKT, 128, 129).transpose(0, 2, 1, 3)).astype(ml_dtypes.bfloat16)

    v2 = np.zeros((H, 128, 4, 129), np.float32)
    for j in range(4):
        v2[:, :, j, :128] = v[:, TRI + j * 128:TRI + (j + 1) * 128]
        v2[:, :, j, 128] = 1.0
    v2 = v2.astype(ml_dtypes.bfloat16)
    v3 = np.zeros((H, 128, 17, 129), np.float32)
    for i in range(16):
        t0 = TRE + i * 128
        v3[:, :, i, :128] = v[:, t0:t0 + 128]
        v3[:, :, i, 128] = 1.0
    v3[:, 0:32, 16, :128] = v[:, TRIR:S]
    v3[:, 0:32, 16, 128] = 1.0
    v3 = v3.astype(ml_dtypes.bfloat16)

    qt2 = np.ascontiguousarray(
        (q[:, TRI:TRIR] * SQ).transpose(0, 2, 1)).astype(ml_dtypes.bfloat16)
    kt2 = np.ascontiguousarray(
        k[:, TRI:TRIR].transpose(0, 2, 1)).astype(ml_dtypes.bfloat16)

    # fused-mask payloads am16 = round(A16*mask + B16), slots (4,20..24)
    M = (rm - 1.0) * 100.0 + REF_SHIFT                    # [512, 2624]
    ref_rt = np.repeat(rt, REF // NCOND, axis=0)
    M[:, TRE:TRI] += (ref_rt - 1.0) * 100.0
    M = np.maximum(M, MASK_CLAMP)
    redux_m = np.maximum((rt - 1.0) * 100.0, MASK_CLAMP)  # [2, 2048]
    am = np.zeros((6, 128, TRI), np.float32)
    am[0, 0:32, TRE:TRI] = A16 * redux_m[0][None, :]      # tile 4 rows: redux
    am[0, 32:64, TRE:TRI] = A16 * redux_m[1][None, :]
    for tt in range(5):                                   # tiles 20..24
        blkm = np.zeros((128, TRI), np.float32)
        kk0 = (20 + tt) * 128 - TRI                       # ref-relative row
        for r in range(128):
            kr = kk0 + r
            if 0 <= kr < REF:
                blkm[r] = A16 * M[kr]
            elif kr >= REF:
                blkm[r] = A16 * MASK_CLAMP                # router+pad rows
        am[1 + tt] = blkm
    am16 = np.round(am + B16).astype(np.int16)
    am16 = np.ascontiguousarray(am16.transpose(1, 0, 2))  # [128, 6, TRI]

    eye = np.eye(128, dtype=np.float16)
    in_maps = []
    for cc in range(8):
        hs = slice(HPC * cc, HPC * (cc + 1))
        in_maps.append({
            "kt": np.ascontiguousarray(kt[hs]),
            "qt": np.ascontiguousarray(qt[hs]),
            "v1": np.ascontiguousarray(v1[hs]),
            "v2": np.ascontiguousarray(v2[hs]),
            "v3": np.ascontiguousarray(v3[hs]),
            "qt2": np.ascontiguousarray(qt2[hs]),
            "kt2": np.ascontiguousarray(kt2[hs]),
            "am16": am16,
            "eye": eye,
        })
    return in_maps


def kernel(query, key, value, ref_mask, routing_map, **_ignored):
    import jax
    if not any(d.platform == "axon" for d in jax.devices()):
        jax.config.update("jax_platforms", "axon,cpu")
    nc = _get_nc()
    in_maps = make_in_maps(query, key, value, ref_mask, routing_map)
    res = run_bass_kernel_spmd(nc, in_maps, core_ids=list(range(8)))
    outs = [res.results[c]["out"] for c in range(8)]      # [3, S, 129] f16
    full = np.concatenate(outs, axis=0).astype(np.float32)
    out = full[:, :, :128] / full[:, :, 128:129]
    return np.ascontiguousarray(out[None].astype(np.float32))


# revision 37
# speedup vs baseline: 1.0047x; 1.0047x over previous
"""AnyStory Flux attention processor on 8 TRN2 NeuronCores.

Sharding: tensor-parallel over heads (24 heads -> 3 per core), no
collectives; the host gathers along the head axis and performs the final
softmax normalization (divide by the ones-column sums) while unsharding.

Device algorithm per head (S=3168 = 512 txt + 64 redux + 2048 img +
512 ref + 32 router; D=128), in ST orientation (k on partitions, q free):

  seg1 (q 0:2624 x k 0:3136+pad): QK and PV in bf16 (fp8 variants were
    measured on host: one-sided e4m3 QK noise gives rel-err 2.9e-2 > 2e-2,
    and fully compensated hi+lo fp8 costs parity with bf16 - no win).
    The softmax exp is split across the two PSUM-capable elementwise
    engines at single k-tile granularity (25 ops per q-block) so the
    PSUM st tiles are one bank each and the QK->exp->PV software
    pipeline can run 6 deep (kills PE stalls on st-buffer recycling):
      ACT tiles (13): true exp -> bf16 p (bias aligns the scale to the
        DVE tiles' Schraudolph factor).
      DVE tiles (12, including every masked tile): Schraudolph exp in
        ONE fused tensor op: i16 = round(184.665*x + 16250.43), bitcast
        = bf16 ~ e^x (+-3%); the additive attention masks ride the same
        instruction as the int16 tensor operand.
  seg2: per-cond ref self-attention, plain bf16 (peaked softmax).
  seg3: router q x [img ; router] keys, bf16, exp on ACT.

  (numerator | denominator) leave PSUM as f16 via ACT/DVE copies
  (alternating) and are DMAed; the host divides while unsharding.
  No max-subtraction anywhere (|logit| <~ 6.6, masks <= 1.5).

DMA orchestration (the cost model serializes HWDGE at 625ns/DMA and
holds the issuing SEQ during a DMA's waits):
  - head-0 inputs are split across two independent DMA paths in strict
    JIT consumption order: kt/qt/am on the SP queue (HWDGE), v1/v2/v3 on
    the gpsimd queue (SWDGE, bypasses HWDGE);
  - output DMAs go on the gpsimd queue (program-order after all gpsimd
    input DMAs) so they never head-of-line-block input dispatch, except
    the final tail blocks which use the then-idle SP queue;
  - head 2's seg2/seg3 run at the very end so the kernel tail is the
    tiny router block;
  - a longer PE warmup (56 matmuls) covers the first-chunk DMA latency
    and finishes the p-state ramp right as real work begins.
"""

import math
import numpy as np
import ml_dtypes
from contextlib import ExitStack

import concourse.bass as bass
import concourse.tile as tile
from concourse import mybir, bacc
from concourse.bass_utils import run_bass_kernel_spmd

# ---- problem constants (hardcoded; kernel.py must be self-contained)
B, H, D = 1, 24, 128
TXT, REDUX, IMG, REF, ROUTER, NCOND = 512, 64, 2048, 512, 32, 2
S = TXT + REDUX + IMG + REF + ROUTER          # 3168
TE = TXT                                       # 512
TRE = TE + REDUX                               # 576
TRI = TRE + IMG                                # 2624
TRIR = TRI + REF                               # 3136
REF_SHIFT = 1.5
SP = 3200                                      # padded key length (25*128)
NKT = SP // 128                                # 25 seg1 k-tiles
HPC = H // 8                                   # heads per core = 3

SQ = 1.0 / math.sqrt(128.0)
A16 = 128.0 / math.log(2.0)                    # 184.664965
B16 = 16250.43                                 # 127*128 - 5.57 (Schraudolph)
# ACT tiles must carry the same scale factor as the Schraudolph tiles:
# bitcast(round(A16*x + B16)) ~ e^x * 2^((B16-16256)/128) * g, E[g]=1.0298
ACT_BIAS = (B16 - 16256.0) / 128.0 * math.log(2.0) + math.log(1.0298)
MASK_CLAMP = -20.0

F32 = mybir.dt.float32
F16 = mybir.dt.float16
BF16 = mybir.dt.bfloat16
I16 = mybir.dt.int16
EXP = mybir.ActivationFunctionType.Exp
COPY = mybir.ActivationFunctionType.Copy
MULT = mybir.AluOpType.mult
ADD = mybir.AluOpType.add

QBLOCKS = [(0, 384), (384, 384), (768, 384), (1152, 384),
           (1536, 384), (1920, 384), (2304, 320)]
# single k-tile work units: (tile, engine, mask_slot); ACT owns tiles
# 0-13 minus 4, DVE owns 14-24 plus 4 (all masked tiles are DVE - the
# fused mask op is a DVE op). Interleave keeps both engines fed; masked
# tiles sit late so their am16 chunks have streamed in.
TILES = [(0, "A", None), (1, "A", None), (2, "A", None), (3, "A", None),
         (14, "D", None), (15, "D", None), (16, "D", None), (17, "D", None),
         (5, "A", None), (6, "A", None), (18, "D", None), (7, "A", None),
         (19, "D", None), (8, "A", None), (4, "D", 0), (9, "A", None),
         (20, "D", 1), (10, "A", None), (21, "D", 2), (11, "A", None),
         (22, "D", 3), (12, "A", None), (23, "D", 4), (13, "A", None),
         (24, "D", 5)]

WARMN = 47          # PE warmup matmuls (64 cols each)
PIPE = 7            # PV lags QK by this many work items
FIN_DEFER = 5       # finalize copies deferred this many items

# head-0 SP-queue (HWDGE) chunk schedule, strict JIT consumption order
H0_SP = [
    ("kt", 0, 512), ("kt", 1792, 2304), ("kt", 512, 1152),
    ("kt", 2304, 2688), ("kt", 1152, 1792), ("kt", 2688, 3200),
    ("am", 0, 2, 0, 384),
    ("am", 2, 4, 0, 384), ("am", 4, 6, 0, 384), ("qt", 384, 768),
    ("am", 0, 2, 384, 1152), ("am", 2, 4, 384, 1152), ("am", 4, 6, 384, 1152),
    ("qt", 768, 1152), ("qt2",), ("kt2",), ("qt", 1152, 1536),
    ("am", 0, 2, 1152, 1920), ("am", 2, 4, 1152, 1920), ("am", 4, 6, 1152, 1920),
    ("qt", 1536, 1920), ("qt", 1920, 2304),
    ("am", 0, 2, 1920, 2624), ("am", 2, 4, 1920, 2624), ("am", 4, 6, 1920, 2624),
    ("qt", 2304, 2624), ("qt", 3136, 3168),
]
# head-0 gpsimd-queue (SWDGE) chunk schedule (v tiles in consumption order;
# qt block 0 rides here so it lands in parallel with SP's first kt chunk)
H0_POOL = [("qt", 0, 384), ("v1", 0, 4), ("v1", 14, 18), ("v1", 4, 9),
           ("v1", 18, 22), ("v1", 9, 14), ("v1", 22, 25), ("v2",), ("v3",),
           ("eye",)]


def _subs(qw):
    out, o = [], 0
    while o < qw:
        w = min(128, qw - o)
        out.append((o, w))
        o += w
    return out


def build_nc():
    nc = bacc.Bacc()
    kt_d = nc.declare_dram_parameter("kt", [HPC, 128, SP], BF16, isOutput=False)
    qt_d = nc.declare_dram_parameter("qt", [HPC, 128, S], BF16, isOutput=False)
    # V pre-tiled partition-major [128, T, 129] (value | ones)
    v1_d = nc.declare_dram_parameter("v1", [HPC, 128, NKT, 129], BF16, isOutput=False)
    v2_d = nc.declare_dram_parameter("v2", [HPC, 128, 4, 129], BF16, isOutput=False)
    v3_d = nc.declare_dram_parameter("v3", [HPC, 128, 17, 129], BF16, isOutput=False)
    qt2_d = nc.declare_dram_parameter("qt2", [HPC, 128, 512], BF16, isOutput=False)
    kt2_d = nc.declare_dram_parameter("kt2", [HPC, 128, 512], BF16, isOutput=False)
    # fused-mask payloads: slots for tiles (4, 20, 21, 22, 23, 24)
    am_d = nc.declare_dram_parameter("am16", [128, 6, TRI], I16, isOutput=False)
    eye_d = nc.declare_dram_parameter("eye", [128, 128], F16, isOutput=False)
    out_d = nc.declare_dram_parameter("out", [HPC, S, 129], F16, isOutput=True)
    out_f = out_d.rearrange("h s d -> (h s) d")

    with ExitStack() as ctx:
        tc = ctx.enter_context(tile.TileContext(nc))
        const = ctx.enter_context(tc.tile_pool(name="const", bufs=1))
        stp = ctx.enter_context(tc.tile_pool(name="st", bufs=6, space="PSUM"))
        accp = ctx.enter_context(tc.tile_pool(name="acc", bufs=2, space="PSUM"))
        ptp = ctx.enter_context(tc.tile_pool(name="pt", bufs=10))
        stgp = ctx.enter_context(tc.tile_pool(name="stg", bufs=10))
        smallp = ctx.enter_context(tc.tile_pool(name="small", bufs=4))

        # warmup weights memset FIRST (on DVE - gpsimd would delay the Pool
        # queue's first input DMA) so PE can start spinning immediately
        warm = const.tile([128, 64], BF16, tag="warm")
        nc.vector.memset(warm[:, :], 0.0)

        am_sb = const.tile([128, 6, TRI], I16, tag="am16")
        bias_sb = const.tile([128, 1], F32, tag="biasA")
        zbias_sb = const.tile([128, 1], F32, tag="biasZ")
        eye_sb = const.tile([128, 128], F16, tag="eye")
        ones_sb = const.tile([128, 1], BF16, tag="ones")
        nc.vector.memset(bias_sb[:, :], ACT_BIAS)
        nc.vector.memset(zbias_sb[:, :], 0.0)
        nc.vector.memset(ones_sb[:, :], 1.0)

        kt_sb, qt_sb, v1_sb = [], [], []
        qt2_sb, kt2_sb, v2_sb, v3_sb = [], [], [], []
        load_fns = []
        for h in range(HPC):
            kt = const.tile([128, SP], BF16, tag=f"kt{h}")
            qt = const.tile([128, S], BF16, tag=f"qt{h}")
            v1 = const.tile([128, NKT, 129], BF16, tag=f"v1{h}")
            qt2 = const.tile([128, 512], BF16, tag=f"qt2{h}")
            kt2 = const.tile([128, 512], BF16, tag=f"kt2{h}")
            v2 = const.tile([128, 4, 129], BF16, tag=f"v2{h}")
            v3 = const.tile([128, 17, 129], BF16, tag=f"v3{h}")
            kt_sb.append(kt); qt_sb.append(qt); v1_sb.append(v1)
            qt2_sb.append(qt2); kt2_sb.append(kt2)
            v2_sb.append(v2); v3_sb.append(v3)

            def load_head(h=h, kt=kt, qt=qt, v1=v1, qt2=qt2, kt2=kt2,
                          v2=v2, v3=v3):
                def chunk(eng, c):
                    kind = c[0]
                    if kind == "kt":
                        eng.dma_start(kt[:, c[1]:c[2]], kt_d[h, :, c[1]:c[2]])
                    elif kind == "qt":
                        eng.dma_start(qt[:, c[1]:c[2]], qt_d[h, :, c[1]:c[2]])
                    elif kind == "am":
                        eng.dma_start(am_sb[:, c[1]:c[2], c[3]:c[4]],
                                      am_d[:, c[1]:c[2], c[3]:c[4]])
                    elif kind == "v1":
                        eng.dma_start(v1[:, c[1]:c[2], :], v1_d[h, :, c[1]:c[2], :])
                    elif kind == "qt2":
                        eng.dma_start(qt2[:, :], qt2_d[h])
                    elif kind == "kt2":
                        eng.dma_start(kt2[:, :], kt2_d[h])
                    elif kind == "v2":
                        eng.dma_start(v2[:, :, :], v2_d[h])
                    elif kind == "v3":
                        eng.dma_start(v3[:, :, :], v3_d[h])
                    elif kind == "eye":
                        eng.dma_start(eye_sb[:, :], eye_d[:, :])

                if h == 0:
                    for c in H0_SP:
                        chunk(nc.sync, c)
                    for c in H0_POOL:
                        chunk(nc.gpsimd, c)
                else:
                    for c in range(4):
                        nc.sync.dma_start(kt[:, c * 800:(c + 1) * 800],
                                          kt_d[h, :, c * 800:(c + 1) * 800])
                    for c in [(0, 792), (792, 1584), (1584, 2376),
                              (2376, 2624), (3136, 3168)]:
                        nc.sync.dma_start(qt[:, c[0]:c[1]], qt_d[h, :, c[0]:c[1]])
                    nc.sync.dma_start(qt2[:, :], qt2_d[h])
                    nc.sync.dma_start(kt2[:, :], kt2_d[h])
                    nc.gpsimd.dma_start(v1[:, :, :], v1_d[h])
                    nc.gpsimd.dma_start(v2[:, :, :], v2_d[h])
                    nc.gpsimd.dma_start(v3[:, :, :], v3_d[h])
            load_fns.append(load_head)

        # ---- PE warmup: spin the Tensor engine to full p-state during the
        # initial DMA wait (the cost model ramps PE clock 0.65->2.4 GHz over
        # ~3us of continuous execution)
        wst = stp.tile([128, 512], F32, tag="st", name="st")
        for wi in range(WARMN):
            nc.tensor.matmul(wst[0:64, 0:64], lhsT=warm[:, :],
                             rhs=warm[:, :], start=True, stop=True)

        # ---- work items (global PIPE-deep software pipeline)
        items = []
        copy_flip = [0]

        def finalize(acc, h, q0, qw, eng=None):
            """Copy acc PSUM -> f16 stage (ACT/DVE alternating), DMA out."""
            if eng is None:
                eng = nc.gpsimd
            subs = _subs(qw)
            stg = stgp.tile([128, 3, 129], F16, tag="stg", name="stg")
            on_act = (copy_flip[0] % 2 == 0)
            copy_flip[0] += 1
            nsub = len(subs)
            if all(wsub == 128 for _, wsub in subs):
                src = acc[:, 0:nsub * 129].rearrange("p (a b) -> p a b", a=nsub)
                if on_act:
                    nc.scalar.activation(stg[:, 0:nsub, :], src, COPY)
                else:
                    nc.vector.tensor_scalar_add(stg[:, 0:nsub, :], src, 0.0)
            else:
                # ragged tail: avoid reading never-written PSUM
                w_full = (nsub - 1) * 129
                lastw = subs[-1][1]
                if nsub > 1:
                    src = acc[:, 0:w_full].rearrange("p (a b) -> p a b", a=nsub - 1)
                    if on_act:
                        nc.scalar.activation(stg[:, 0:nsub - 1, :], src, COPY)
                    else:
                        nc.vector.tensor_scalar_add(stg[:, 0:nsub - 1, :], src, 0.0)
                if on_act:
                    nc.scalar.activation(stg[0:lastw, nsub - 1, :],
                                         acc[0:lastw, w_full:w_full + 129], COPY)
                else:
                    nc.vector.tensor_scalar_add(stg[0:lastw, nsub - 1, :],
                                                acc[0:lastw, w_full:w_full + 129], 0.0)
            r0 = h * S + q0
            nfull = qw // 128
            if nfull:
                dst = out_f[r0:r0 + nfull * 128, :].rearrange(
                    "(si p) d -> p si d", si=nfull)
                eng.dma_start(dst, stg[:, 0:nfull, :])
            if qw % 128:
                lw = qw % 128
                eng.dma_start(out_f[r0 + nfull * 128:r0 + qw, :],
                              stg[0:lw, nfull, :])

        for h in range(HPC):
            load_fns[h]()
            kt, qt, v1 = kt_sb[h], qt_sb[h], v1_sb[h]
            head_items = []

            for qbi, (q0, qw) in enumerate(QBLOCKS):
                subs = _subs(qw)
                blk = {}

                def qk1(st, t, kt=kt, qt=qt, q0=q0, qw=qw):
                    nc.tensor.matmul(
                        st[:, 0:qw],
                        lhsT=kt[:, t * 128:(t + 1) * 128],
                        rhs=qt[:, q0:q0 + qw],
                        start=True, stop=True)

                def ex1(st, t, eng, mslot, q0=q0, qw=qw):
                    pt = ptp.tile([128, 512], BF16, tag="pt", name="pt")
                    if eng == "A":
                        nc.scalar.activation(pt[:, 0:qw], st[:, 0:qw],
                                             EXP, bias=bias_sb[:, :], scale=1.0)
                    elif mslot is None:
                        nc.vector.tensor_scalar(
                            pt[:, 0:qw].bitcast(I16),
                            st[:, 0:qw], A16, B16, MULT, ADD)
                    else:
                        nc.vector.scalar_tensor_tensor(
                            pt[:, 0:qw].bitcast(I16),
                            st[:, 0:qw], A16,
                            am_sb[:, mslot, q0:q0 + qw], MULT, ADD)
                    return pt

                def pv1(pt, t, last, h=h, v1=v1, q0=q0, qw=qw, subs=subs, blk=blk):
                    if "acc" not in blk:
                        blk["acc"] = accp.tile([128, 512], F32, tag="acc", name="acc")
                        blk["n"] = 0
                    acc = blk["acc"]
                    for si, (qs0, qsw) in enumerate(subs):
                        nc.tensor.matmul(
                            acc[0:qsw, si * 129:si * 129 + 129],
                            lhsT=pt[:, qs0:qs0 + qsw],
                            rhs=v1[:, t, :],
                            start=(blk["n"] == 0),
                            stop=(last and si == len(subs) - 1))
                        blk["n"] += 1
                    if last:
                        # last block of the last head drains via the idle SP
                        # queue so the kernel tail isn't swdge-latency-bound
                        eng = nc.sync if (h == HPC - 1 and q0 + qw == TRI) else None
                        return lambda: finalize(acc, h, q0, qw, eng)

                for ti, (t, eng, mslot) in enumerate(TILES):
                    last = (ti == len(TILES) - 1)
                    head_items.append((
                        (lambda st, t=t, f=qk1: f(st, t)),
                        (lambda st, t=t, e=eng, m=mslot, f=ex1: f(st, t, e, m)),
                        (lambda pt, t=t, l=last, f=pv1: f(pt, t, l)),
                    ))

            # ===== seg2: per-cond ref self-attention (bf16) =====
            seg23_items = []
            last_head = (h == HPC - 1)
            for c in range(NCOND):
                b0 = 256 * c

                def qk2(st, h=h, b0=b0):
                    for j in range(2):
                        nc.tensor.matmul(
                            st[:, j * 256:(j + 1) * 256],
                            lhsT=kt2_sb[h][:, b0 + j * 128:b0 + (j + 1) * 128],
                            rhs=qt2_sb[h][:, b0:b0 + 256],
                            start=True, stop=True)

                def ex2(st, h=h):
                    pt = smallp.tile([128, 512], BF16, tag="pt2", name="pt2")
                    nc.scalar.activation(pt[:, 0:512], st[:, 0:512],
                                         EXP, bias=zbias_sb[:, :], scale=1.0)
                    return pt

                def pv2(pt, h=h, b0=b0, c=c, lh=last_head):
                    acc = accp.tile([128, 512], F32, tag="acc", name="acc")
                    for j in range(2):
                        for si in range(2):
                            nc.tensor.matmul(
                                acc[0:128, si * 129:si * 129 + 129],
                                lhsT=pt[:, j * 256 + si * 128:j * 256 + (si + 1) * 128],
                                rhs=v2_sb[h][:, 2 * c + j, :],
                                start=(j == 0 and si == 0),
                                stop=(j == 1 and si == 1))
                    return lambda: finalize(acc, h, TRI + b0, 256)

                seg23_items.append((qk2, ex2, pv2))

            # ===== seg3: router queries =====
            def qk3(st, h=h, kt=kt, qt=qt):
                # st is a pair (img-part tile, router-self tile)
                st_a, st_b = st
                for i in range(16):
                    nc.tensor.matmul(
                        st_a[:, i * 32:(i + 1) * 32],
                        lhsT=kt[:, TRE + i * 128:TRE + (i + 1) * 128],
                        rhs=qt[:, TRIR:TRIR + 32],
                        start=(i == 0), stop=(i == 15))
                nc.tensor.matmul(
                    st_b[0:32, 0:32],
                    lhsT=kt[:, TRIR:TRIR + 32],
                    rhs=qt[:, TRIR:TRIR + 32],
                    start=True, stop=True)

            def ex3(st, h=h):
                st_a, st_b = st
                pt = smallp.tile([128, 512], BF16, tag="pt3", name="pt3")
                ptb = smallp.tile([128, 32], BF16, tag="pt3b", name="pt3b")
                nc.scalar.activation(pt[:, 0:512], st_a[:, 0:512],
                                     EXP, bias=zbias_sb[:, :], scale=1.0)
                nc.scalar.activation(ptb[0:32, 0:32], st_b[0:32, 0:32],
                                     EXP, bias=zbias_sb[0:32, :], scale=1.0)
                return (pt, ptb)

            def pv3(pts, h=h, lh=last_head):
                # transposed PV: v stationary, p moving -> numerator lands as
                # [128 d, 32 q] (544 streamed cols instead of 17*129); the
                # denominator comes from 1-col ones-contractions; a PE
                # transpose via the identity then restores [32 q, 128 d].
                # num (f32 cols 0:32), den (col 40) and the f16 transpose
                # scratch (f32 cols 128:192) share one PSUM bank: the bank is
                # zeroed by the first matmul's start, and later start=False
                # writes to untouched regions land on hardware-zeroed bytes.
                pt, ptb = pts
                acc = accp.tile([128, 512], F32, tag="acc", name="acc")
                for i in range(16):
                    nc.tensor.matmul(
                        acc[0:128, 0:32],
                        lhsT=v3_sb[h][:, i, 0:128],
                        rhs=pt[:, i * 32:(i + 1) * 32],
                        start=(i == 0), stop=False)
                nc.tensor.matmul(
                    acc[0:128, 0:32],
                    lhsT=v3_sb[h][0:32, 16, 0:128],
                    rhs=ptb[0:32, 0:32],
                    start=False, stop=False, skip_group_check=True)
                for i in range(16):
                    nc.tensor.matmul(
                        acc[0:32, 40:41],
                        lhsT=pt[:, i * 32:(i + 1) * 32],
                        rhs=ones_sb[:, 0:1],
                        start=False, stop=False, skip_group_check=True)
                nc.tensor.matmul(
                    acc[0:32, 40:41],
                    lhsT=ptb[0:32, 0:32],
                    rhs=ones_sb[0:32, 0:1],
                    start=False, stop=True, skip_group_check=True)

                def fin(acc=acc, h=h, lh=lh):
                    tb = smallp.tile([128, 32], F16, tag="tb", name="tb")
                    nc.scalar.activation(tb[:, 0:32], acc[0:128, 0:32], COPY)
                    ot = acc[:, 128:192].bitcast(F16)      # [128, 128] f16
                    nc.tensor.matmul(ot[0:32, 0:128], lhsT=tb[:, 0:32],
                                     rhs=eye_sb[:, 0:128], is_transpose=True,
                                     start=False, stop=True,
                                     skip_group_check=True)
                    stg = stgp.tile([128, 3, 129], F16, tag="stg", name="stg")
                    nc.vector.tensor_scalar_add(stg[0:32, 0, 0:128],
                                                ot[0:32, 0:128], 0.0)
                    nc.scalar.activation(stg[0:32, 0, 128:129],
                                         acc[0:32, 40:41], COPY)
                    r0 = h * S + TRIR
                    eng = nc.sync if lh else nc.gpsimd
                    eng.dma_start(out_f[r0:r0 + 32, :], stg[0:32, 0, :])
                return fin

            seg23_items.append((qk3, ex3, pv3, "seg3"))
            if last_head:
                # tail: weave seg2 into the last q-block so its exps overlap
                # seg1 PE work; seg3 runs dead last so the kernel ends on the
                # tiny router block's DMA
                head_items.insert(162, seg23_items[0])
                head_items.insert(170, seg23_items[1])
                head_items.append(seg23_items[2])
            else:
                # splice into the middle of the head's stream so their small
                # bursty windows don't cluster at head boundaries
                for i, it in enumerate(seg23_items):
                    head_items.insert(58 + i * 25, it)
            items.extend(head_items)

        # ---- run the global pipeline; finalize copies are deferred a few
        # items so they never sit in an exp engine's queue ahead of work the
        # PE pipeline depends on
        pending, fins = [], []
        idx = 0
        nitems = len(items)
        for it in items:
            fqk, fex, fpv = it[0], it[1], it[2]
            is_seg3 = len(it) > 3
            while fins and fins[0][1] <= idx:
                fins.pop(0)[0]()
            if is_seg3:
                st = (stp.tile([128, 512], F32, tag="st", name="st"),
                      stp.tile([128, 512], F32, tag="st", name="st"))
            else:
                st = stp.tile([128, 512], F32, tag="st", name="st")
            fqk(st)
            while len(pending) >= PIPE:
                fin = pending.pop(0)()
                if fin is not None:
                    fins.append((fin, idx + FIN_DEFER))
            pt = fex(st)
            pending.append(lambda f=fpv, p=pt: f(p))
            idx += 1
        while pending:
            while fins and fins[0][1] <= idx:
                fins.pop(0)[0]()
            fin = pending.pop(0)()
            if fin is not None:
                fins.append((fin, idx))
            idx += 1
        while fins:
            fins.pop(0)[0]()

    nc.compile()
    return nc


_NC_CACHE = None


def _get_nc():
    global _NC_CACHE
    if _NC_CACHE is None:
        _NC_CACHE = build_nc()
    return _NC_CACHE


def make_in_maps(query, key, value, ref_mask, routing_map):
    q = np.asarray(query, np.float32)[0]                  # [24, S, 128]
    k = np.asarray(key, np.float32)[0]
    v = np.asarray(value, np.float32)[0]
    rm = np.asarray(ref_mask, np.float32)[0]              # [512, 2624]
    rt = np.asarray(routing_map, np.float32)[0]           # [2, 2048]

    qt = np.ascontiguousarray(
        (q * SQ).transpose(0, 2, 1)).astype(ml_dtypes.bfloat16)   # [24,128,S]
    ktf = np.zeros((H, 128, SP), np.float32)
    ktf[:, :, :S] = k.transpose(0, 2, 1)
    kt = ktf.astype(ml_dtypes.bfloat16)

    # V (+ones) pre-tiled partition-major [128, T, 129]
    vv = np.zeros((H, SP, 129), np.float32)
    vv[:, :S, :128] = v
    vv[:, :TRIR, 128] = 1.0                               # ones: seg1 keys only
    vv[:, 24 * 128 + 64:] = 0.0                           # router+pad rows
    v1 = np.ascontiguousarray(
        vv.reshape(H, NKT, 128, 129).transpose(0, 2, 1, 3)).astype(ml_dtypes.bfloat16)

    v2 = np.zeros((H, 128, 4, 129), np.float32)
    for j in range(4):
        v2[:, :, j, :128] = v[:, TRI + j * 128:TRI + (j + 1) * 128]
        v2[:, :, j, 128] = 1.0
    v2 = v2.astype(ml_dtypes.bfloat16)
    v3 = np.zeros((H, 128, 17, 129), np.float32)
    for i in range(16):
        t0 = TRE + i * 128
        v3[:, :, i, :128] = v[:, t0:t0 + 128]
        v3[:, :, i, 128] = 1.0
    v3[:, 0:32, 16, :128] = v[:, TRIR:S]
    v3[:, 0:32, 16, 128] = 1.0
    v3 = v3.astype(ml_dtypes.bfloat16)

    qt2 = np.ascontiguousarray(
        (q[:, TRI:TRIR] * SQ).transpose(0, 2, 1)).astype(ml_dtypes.bfloat16)
    kt2 = np.ascontiguousarray(
        k[:, TRI:TRIR].transpose(0, 2, 1)).astype(ml_dtypes.bfloat16)

    # fused-mask payloads am16 = round(A16*mask + B16), slots (4,20..24)
    M = (rm - 1.0) * 100.0 + REF_SHIFT                    # [512, 2624]
    ref_rt = np.repeat(rt, REF // NCOND, axis=0)
    M[:, TRE:TRI] += (ref_rt - 1.0) * 100.0
    M = np.maximum(M, MASK_CLAMP)
    redux_m = np.maximum((rt - 1.0) * 100.0, MASK_CLAMP)  # [2, 2048]
    am = np.zeros((6, 128, TRI), np.float32)
    am[0, 0:32, TRE:TRI] = A16 * redux_m[0][None, :]      # tile 4 rows: redux
    am[0, 32:64, TRE:TRI] = A16 * redux_m[1][None, :]
    for tt in range(5):                                   # tiles 20..24
        blkm = np.zeros((128, TRI), np.float32)
        kk0 = (20 + tt) * 128 - TRI                       # ref-relative row
        for r in range(128):
            kr = kk0 + r
            if 0 <= kr < REF:
                blkm[r] = A16 * M[kr]
            elif kr >= REF:
                blkm[r] = A16 * MASK_CLAMP                # router+pad rows
        am[1 + tt] = blkm
    am16 = np.round(am + B16).astype(np.int16)
    am16 = np.ascontiguousarray(am16.transpose(1, 0, 2))  # [128, 6, TRI]

    eye = np.eye(128, dtype=np.float16)
    in_maps = []
    for cc in range(8):
        hs = slice(HPC * cc, HPC * (cc + 1))
        in_maps.append({
            "kt": np.ascontiguousarray(kt[hs]),
            "qt": np.ascontiguousarray(qt[hs]),
            "v1": np.ascontiguousarray(v1[hs]),
            "v2": np.ascontiguousarray(v2[hs]),
            "v3": np.ascontiguousarray(v3[hs]),
            "qt2": np.ascontiguousarray(qt2[hs]),
            "kt2": np.ascontiguousarray(kt2[hs]),
            "am16": am16,
            "eye": eye,
        })
    return in_maps


def kernel(query, key, value, ref_mask, routing_map, **_ignored):
    import jax
    if not any(d.platform == "axon" for d in jax.devices()):
        jax.config.update("jax_platforms", "axon,cpu")
    nc = _get_nc()
    in_maps = make_in_maps(query, key, value, ref_mask, routing_map)
    res = run_bass_kernel_spmd(nc, in_maps, core_ids=list(range(8)))
    outs = [res.results[c]["out"] for c in range(8)]      # [3, S, 129] f16
    full = np.concatenate(outs, axis=0).astype(np.float32)
    out = full[:, :, :128] / full[:, :, 128:129]
    return np.ascontiguousarray(out[None].astype(np.float32))
